# revision 13
# baseline (speedup 1.0000x reference)
"""3x3 neighborhood cosine-similarity sum (minus self) on 8 TRN2 NeuronCores.

Input:  input_image [1024, 1024, 1, 128] float32  (H, W, 1, C)
Output: sim [1024, 1024] float32

Algorithm per pixel: sim = <xn, BoxSum3x3(xn)> - 1, xn = x / max(||x||, eps).

Design (v3):
 - Host casts x to bf16 and pre-gathers an OVERLAPPED chunk layout:
   chunk j (j=0..8) holds w = 126*j + p - 1 for partition p=0..127
   (chunks overlap by 2 columns; out-of-range w zero; valid outputs are
   p in [1,126] per chunk).  Horizontal 3-tap sum becomes one constant
   tridiagonal matmul per chunk with no cross-chunk terms.
 - Rows processed in blocks of R=4 to amortize DVE instruction overhead:
     sq   : one TT mult (x*x) over [128, R*9*128] bf16
     ss   : in-place halving-tree adds (7 levels) -> [128, R, 9] f32
     sqrt : ACT (batched, +1e-16 bias), recip: DVE (batched)
     xn   : per-(row,chunk) tensor_scalar (bf16, per-partition scalar)
     PE   : band matmuls accumulate T@xn(h) into PSUM S(h-1),S(h),S(h+1)
            (vertical fold in PSUM; close-early ordering; 2 live tiles)
     evac : ACT copy PSUM->SBUF bf16 into the block's S_sb slice
     dot  : one TT mult (xn*S) per block + halving tree -> sim [128,R,9]
 - Output written per block as [rows, 128, 9] f32; host extracts the
   valid (p, j) window and reassembles [H, W].

Sharding: H rows split 128/core across 8 cores, 1 zero halo row each side.
"""

import os
import sys

import numpy as np
import ml_dtypes

for _p in ("/opt/trn_rl_repo",):
    if _p not in sys.path:
        sys.path.insert(0, _p)

import concourse.bass as bass
import concourse.bacc as bacc
import concourse.mybir as mybir
import concourse.tile as tile
from concourse.bass_utils import run_bass_kernel_spmd

F32 = mybir.dt.float32
BF16 = mybir.dt.bfloat16
ALU = mybir.AluOpType
ACTF = mybir.ActivationFunctionType


def _register_sq_add_sq():
    """Custom DVE op: out = in0^2 + in1^2 (fuses the square pass with the
    first halving-tree level of the sum-of-squares reduction)."""
    import concourse.dve_ops as dvo
    from concourse.dve_spec import Spec, Src0, Src1, sq

    name = "SQ_ADD_SQ_ANT"
    for op in dvo.OPS:
        if op.name == name:
            return op
    spec = Spec(
        body=sq(Src0) + sq(Src1),
        reference=lambda in0, in1, s0, s1, imm2: (
            np.asarray(in0, np.float32) ** 2 + np.asarray(in1, np.float32) ** 2
        ),
    )
    dvo._SUB_OPCODE_FOR_NAME[name] = max(dvo._SUB_OPCODE_FOR_NAME.values()) + 1
    op = dvo.DveOp(
        name,
        spec,
        subdim=False,
        uops_sha={"v3": "cd4bd6e1c27efd14", "v4": "121e32d8332f5047"},
    )
    dvo.OPS.append(op)
    dvo.CUSTOM_DVE_SPECS[name] = spec
    return op


SQOP = _register_sq_add_sq()

H, W, C = 1024, 1024, 128
NCORES = 8
ROWS_PER_CORE = H // NCORES          # 128
NJ = 9                               # overlapped w-chunks, stride 126
WSTRIDE = 126
N_IN = ROWS_PER_CORE + 2             # 130 rows incl zero halos
RB = 8                               # rows per batch block
XN_ACT_CHUNKS = 4                    # xn chunks offloaded to the Scalar engine


def build_consts():
    t = np.zeros((128, 128), np.float32)
    for k in range(128):
        for m in (k - 1, k, k + 1):
            if 0 <= m < 128:
                t[k, m] = 1.0
    return t.astype(ml_dtypes.bfloat16)


def build_bass(n_out_rows=ROWS_PER_CORE):
    n_in = n_out_rows + 2
    nc = bacc.Bacc(None, target_bir_lowering=False)
    x_dram = nc.declare_dram_parameter("x", [n_in, 128, NJ * C], BF16, isOutput=False)
    band_dram = nc.declare_dram_parameter("band", [128, 128], BF16, isOutput=False)
    out_dram = nc.declare_dram_parameter(
        "out", [n_out_rows, 128, NJ], F32, isOutput=True
    )

    # input row blocks: [h0, h1) ranges of size <= RB
    blocks = [(h0, min(h0 + RB, n_in)) for h0 in range(0, n_in, RB)]

    with tile.TileContext(nc) as tc:
        with (
            tc.tile_pool(name="consts", bufs=1) as cpool,
            tc.tile_pool(name="xin", bufs=2) as xpool,
            tc.tile_pool(name="sq", bufs=2) as sqpool,
            tc.tile_pool(name="norm", bufs=3) as npool,
            tc.tile_pool(name="xn", bufs=3) as xnpool,
            tc.tile_pool(name="sb", bufs=2) as sbpool,
            tc.tile_pool(name="pd", bufs=2) as pdpool,
            tc.tile_pool(name="sim", bufs=2) as simpool,
            tc.tile_pool(name="psum", bufs=2, space="PSUM") as psumpool,
        ):
            band = cpool.tile([128, 128], BF16, tag="band")
            nc.sync.dma_start(band[:], band_dram[:])
            eps_bias = cpool.tile([128, 1], F32, tag="eps")
            nc.gpsimd.memset(eps_bias[:], 1e-16)

            # ring state
            xn_tiles = {}       # block index -> xn tile [128, R, NJ, C]
            s_psum = [None] * (n_in + 2)
            sb_tiles = {}       # block index -> S_sb tile [128, R, NJ, C]

            SPLITS = [(0, 4), (4, 8), (8, 9)]

            def band_matmuls(S, xn_t, start, stop):
                for j0, j1 in SPLITS:
                    nc.tensor.matmul(
                        S[:, j0:j1, :], band[:], xn_t[:, j0:j1, :],
                        start=start, stop=stop,
                    )

            def tree_levels(big3, wdt, out_ap, final_scalar=None):
                """big3 [128, n, wdt] bf16 (3D view), in-place halving tree;
                final level writes `out_ap` [128, n] sums (+final_scalar)."""
                while wdt > 2:
                    h = wdt // 2
                    nc.vector.tensor_tensor(
                        big3[:, :, 0:h], big3[:, :, 0:h], big3[:, :, h:wdt],
                        ALU.add,
                    )
                    wdt = h
                if final_scalar is None:
                    nc.vector.tensor_tensor(
                        out_ap, big3[:, :, 0], big3[:, :, 1], ALU.add
                    )
                else:
                    nc.vector.scalar_tensor_tensor(
                        out_ap, big3[:, :, 0], final_scalar, big3[:, :, 1],
                        ALU.add, ALU.add,
                    )

            def emit_dot_batch(b, h0, h1):
                """dot for rows r in [h0, h1) ∩ [1, n_out]; uses xn block b
                and its S_sb tile; writes sim rows and DMAs out."""
                r0 = max(h0, 1)
                r1 = min(h1, n_out_rows + 1)  # r <= 128
                if r0 >= r1:
                    return
                k0 = r0 - h0
                k1 = r1 - h0
                n0, n1 = k0 * NJ, k1 * NJ
                xbf = xn_tiles[b].rearrange("p r j c -> p (r j) c")
                sbf = sb_tiles.pop(b).rearrange("p r j c -> p (r j) c")
                pd = pdpool.tile([128, RB * NJ, C], BF16, tag="pd", name="pd")
                nc.vector.tensor_tensor(
                    pd[:, n0:n1], xbf[:, n0:n1], sbf[:, n0:n1], ALU.mult
                )
                simo = simpool.tile([128, RB, NJ], F32, tag="simo", name="simo")
                simof = simo.rearrange("p r j -> p (r j)")
                # final level fuses the "- 1" (self-similarity) term
                tree_levels(pd[:, n0:n1], C, simof[:, n0:n1], final_scalar=-1.0)
                nc.sync.dma_start(
                    out_dram[r0 - 1 : r1 - 1].rearrange("r p j -> p r j"),
                    simo[:, k0:k1],
                )

            for b, (h0, h1) in enumerate(blocks):
                R = h1 - h0

                xt = xpool.tile([128, RB, NJ, C], BF16, tag="xt", name="xt")
                nc.sync.dma_start(
                    xt[:, 0:R], x_dram[h0:h1].rearrange("r p f -> p r f")
                )

                # ---- fused square + first tree level -> sq [128, R*NJ, 64]
                n = R * NJ
                xtf = xt.rearrange("p r j c -> p (r j) c")
                sq = sqpool.tile([128, RB * NJ, C // 2], BF16, tag="sq", name="sq")
                nc.vector._custom_dve(
                    SQOP,
                    out=sq[:, 0:n],
                    in0=xtf[:, 0:n, 0 : C // 2],
                    in1=xtf[:, 0:n, C // 2 : C],
                )
                ssr = npool.tile([128, RB, NJ], F32, tag="ssr", name="ssr")
                ssrf = ssr.rearrange("p r j -> p (r j)")
                tree_levels(sq[:, 0:n], C // 2, ssrf[:, 0:n])

                # ---- inv = 1/sqrt(ss + 1e-16), batched
                snorm = npool.tile([128, RB, NJ], F32, tag="snorm", name="snorm")
                nc.scalar.activation(
                    snorm[:, 0:R], ssr[:, 0:R], ACTF.Sqrt, bias=eps_bias[:]
                )
                sinv = npool.tile([128, RB, NJ], F32, tag="sinv", name="sinv")
                nc.vector.reciprocal(sinv[:, 0:R], snorm[:, 0:R])

                # ---- xn chunks
                xnb = xnpool.tile([128, RB, NJ, C], BF16, tag="xn", name="xn")
                xn_tiles[b] = xnb
                sb_tiles[b] = sbpool.tile(
                    [128, RB, NJ, C], BF16, tag="sbt", name="sbt"
                )

                for k in range(R):
                    h = h0 + k
                    for j in range(NJ):
                        if j >= NJ - XN_ACT_CHUNKS:
                            nc.scalar.activation(
                                xnb[:, k, j, :],
                                xt[:, k, j, :],
                                ACTF.Copy,
                                bias=0.0,
                                scale=sinv[:, k, j : j + 1],
                            )
                        else:
                            nc.vector.tensor_scalar(
                                xnb[:, k, j, :],
                                xt[:, k, j, :],
                                sinv[:, k, j : j + 1],
                                None,
                                ALU.mult,
                            )
                    xnt = xnb[:, k]

                    # ---- PE vertical-fold band matmuls (close-early order)
                    r = h - 1
                    if 1 <= r <= n_in - 2:
                        band_matmuls(s_psum[r], xnt, start=False, stop=True)
                        # evac into the owning block's S_sb slice
                        rb = r // RB
                        kk = r - rb * RB
                        nc.scalar.activation(
                            sb_tiles[rb][:, kk], s_psum[r][:], ACTF.Copy
                        )
                        s_psum[r] = None
                        # if this closed the last (valid) row of block rb,
                        # emit its dot batch
                        bh0, bh1 = blocks[rb]
                        if r == min(bh1 - 1, n_out_rows):
                            emit_dot_batch(rb, bh0, bh1)

                    if 1 <= h <= n_in - 2:
                        band_matmuls(s_psum[h], xnt, start=False, stop=False)

                    if 1 <= h + 1 <= n_in - 2:
                        assert s_psum[h + 1] is None
                        s_psum[h + 1] = psumpool.tile(
                            [128, NJ, C], F32, tag="S", name="S"
                        )
                        band_matmuls(s_psum[h + 1], xnt, start=True, stop=False)

                # free xn of block b-2 (consumed by its dot batch by now)
                xn_tiles.pop(b - 2, None)

    nc.compile()
    return nc


def _overlap_gather_full(x):
    xb = x.astype(ml_dtypes.bfloat16)
    WPAD = WSTRIDE * (NJ - 1) + 128 + 8
    xw = np.zeros((H, WPAD, C), ml_dtypes.bfloat16)
    xw[:, 1 : 1 + W] = xb
    p = np.arange(128)[:, None]
    j = np.arange(NJ)[None, :]
    widx = WSTRIDE * j + p
    xov = xw[:, widx, :]  # [H, 128, NJ, C]
    out = np.zeros((H + 2, 128, NJ * C), ml_dtypes.bfloat16)
    out[1 : H + 1] = xov.reshape(H, 128, NJ * C)
    return out


def shard_inputs(input_image):
    x = np.asarray(input_image).reshape(H, W, C).astype(np.float32, copy=False)
    xpad = _overlap_gather_full(x)
    band = build_consts()
    in_maps = []
    for core in range(NCORES):
        lo = core * ROWS_PER_CORE
        shard = np.ascontiguousarray(xpad[lo : lo + N_IN])
        in_maps.append({"x": shard, "band": band})
    return in_maps


def unshard_output(results):
    """results[i]['out'] [128, 128, NJ] (rows, p, j) -> [H, W] f32."""
    w = np.arange(W)
    jw = np.minimum(w // WSTRIDE, NJ - 1)
    pw = w - WSTRIDE * jw + 1
    out = np.empty((H, W), np.float32)
    for core in range(NCORES):
        st = np.asarray(results[core]["out"])  # [rows, 128, NJ]
        h0 = core * ROWS_PER_CORE
        out[h0 : h0 + ROWS_PER_CORE] = st[:, pw, jw]
    return out


_NC_CACHE = {}


def get_nc():
    if "nc" not in _NC_CACHE:
        _NC_CACHE["nc"] = build_bass()
    return _NC_CACHE["nc"]


def kernel(input_image):
    nc = get_nc()
    in_maps = shard_inputs(input_image)
    res = run_bass_kernel_spmd(nc, in_maps, list(range(NCORES)))
    return unshard_output(res.results)


if __name__ == "__main__":
    rng = np.random.default_rng(0)
    x = rng.standard_normal((H, W, 1, C), dtype=np.float32)
    out = kernel(x)
    print(out.shape, out.dtype, out[:2, :4])


# revision 16
# speedup vs baseline: 1.0446x; 1.0446x over previous
"""3x3 neighborhood cosine-similarity sum (minus self) on 8 TRN2 NeuronCores.

Input:  input_image [1024, 1024, 1, 128] float32  (H, W, 1, C)
Output: sim [1024, 1024] float32

Algorithm per pixel: sim = <xn, BoxSum3x3(xn)> - 1, xn = x / max(||x||, eps).

Design (v3):
 - Host casts x to bf16 and pre-gathers an OVERLAPPED chunk layout:
   chunk j (j=0..8) holds w = 126*j + p - 1 for partition p=0..127
   (chunks overlap by 2 columns; out-of-range w zero; valid outputs are
   p in [1,126] per chunk).  Horizontal 3-tap sum becomes one constant
   tridiagonal matmul per chunk with no cross-chunk terms.
 - Rows processed in blocks of R=4 to amortize DVE instruction overhead:
     sq   : one TT mult (x*x) over [128, R*9*128] bf16
     ss   : in-place halving-tree adds (7 levels) -> [128, R, 9] f32
     sqrt : ACT (batched, +1e-16 bias), recip: DVE (batched)
     xn   : per-(row,chunk) tensor_scalar (bf16, per-partition scalar)
     PE   : band matmuls accumulate T@xn(h) into PSUM S(h-1),S(h),S(h+1)
            (vertical fold in PSUM; close-early ordering; 2 live tiles)
     evac : ACT copy PSUM->SBUF bf16 into the block's S_sb slice
     dot  : one TT mult (xn*S) per block + halving tree -> sim [128,R,9]
 - Output written per block as [rows, 128, 9] f32; host extracts the
   valid (p, j) window and reassembles [H, W].

Sharding: H rows split 128/core across 8 cores, 1 zero halo row each side.
"""

import os
import sys

import numpy as np
import ml_dtypes

for _p in ("/opt/trn_rl_repo",):
    if _p not in sys.path:
        sys.path.insert(0, _p)

import concourse.bass as bass
import concourse.bacc as bacc
import concourse.mybir as mybir
import concourse.tile as tile
from concourse.bass_utils import run_bass_kernel_spmd

F32 = mybir.dt.float32
BF16 = mybir.dt.bfloat16
ALU = mybir.AluOpType
ACTF = mybir.ActivationFunctionType


def _register_sq_add_sq():
    """Custom DVE op: out = in0^2 + in1^2 (fuses the square pass with the
    first halving-tree level of the sum-of-squares reduction)."""
    import concourse.dve_ops as dvo
    from concourse.dve_spec import Spec, Src0, Src1, sq

    name = "SQ_ADD_SQ_ANT"
    for op in dvo.OPS:
        if op.name == name:
            return op
    spec = Spec(
        body=sq(Src0) + sq(Src1),
        reference=lambda in0, in1, s0, s1, imm2: (
            np.asarray(in0, np.float32) ** 2 + np.asarray(in1, np.float32) ** 2
        ),
    )
    dvo._SUB_OPCODE_FOR_NAME[name] = max(dvo._SUB_OPCODE_FOR_NAME.values()) + 1
    op = dvo.DveOp(
        name,
        spec,
        subdim=False,
        uops_sha={"v3": "cd4bd6e1c27efd14", "v4": "121e32d8332f5047"},
    )
    dvo.OPS.append(op)
    dvo.CUSTOM_DVE_SPECS[name] = spec
    return op


SQOP = _register_sq_add_sq()

H, W, C = 1024, 1024, 128
NCORES = 8
ROWS_PER_CORE = H // NCORES          # 128
NJ = 9                               # overlapped w-chunks, stride 126
WSTRIDE = 126
N_IN = ROWS_PER_CORE + 2             # 130 rows incl zero halos
RB = 6                               # rows per batch block
XN_ACT_CHUNKS = 4                    # xn chunks offloaded to the Scalar engine


def build_consts():
    t = np.zeros((128, 128), np.float32)
    for k in range(128):
        for m in (k - 1, k, k + 1):
            if 0 <= m < 128:
                t[k, m] = 1.0
    return t.astype(ml_dtypes.bfloat16)


def build_bass(n_out_rows=ROWS_PER_CORE):
    n_in = n_out_rows + 2
    nc = bacc.Bacc(None, target_bir_lowering=False)
    x_dram = nc.declare_dram_parameter("x", [n_in, 128, NJ * C], BF16, isOutput=False)
    band_dram = nc.declare_dram_parameter("band", [128, 128], BF16, isOutput=False)
    out_dram = nc.declare_dram_parameter(
        "out", [n_out_rows, 128, NJ], F32, isOutput=True
    )

    # input row blocks: [h0, h1) ranges of size <= RB
    blocks = [(h0, min(h0 + RB, n_in)) for h0 in range(0, n_in, RB)]

    with tile.TileContext(nc) as tc:
        with (
            tc.tile_pool(name="consts", bufs=1) as cpool,
            tc.tile_pool(name="xin", bufs=3) as xpool,
            tc.tile_pool(name="sq", bufs=2) as sqpool,
            tc.tile_pool(name="norm", bufs=4) as npool,
            tc.tile_pool(name="xn", bufs=3) as xnpool,
            tc.tile_pool(name="sb", bufs=3) as sbpool,
            tc.tile_pool(name="pd", bufs=3) as pdpool,
            tc.tile_pool(name="sim", bufs=3) as simpool,
            tc.tile_pool(name="psum", bufs=2, space="PSUM") as psumpool,
        ):
            band = cpool.tile([128, 128], BF16, tag="band")
            nc.sync.dma_start(band[:], band_dram[:])
            eps_bias = cpool.tile([128, 1], F32, tag="eps")
            nc.gpsimd.memset(eps_bias[:], 1e-16)

            # ring state
            xn_tiles = {}       # block index -> xn tile [128, R, NJ, C]
            s_psum = [None] * (n_in + 2)
            sb_tiles = {}       # block index -> S_sb tile [128, R, NJ, C]

            SPLITS = [(0, 4), (4, 8), (8, 9)]

            def band_matmuls(S, xn_t, start, stop):
                for j0, j1 in SPLITS:
                    nc.tensor.matmul(
                        S[:, j0:j1, :], band[:], xn_t[:, j0:j1, :],
                        start=start, stop=stop,
                    )

            def tree_levels(big3, wdt, out_ap, final_scalar=None):
                """big3 [128, n, wdt] bf16 (3D view), in-place halving tree;
                final level writes `out_ap` [128, n] sums (+final_scalar)."""
                while wdt > 2:
                    h = wdt // 2
                    nc.vector.tensor_tensor(
                        big3[:, :, 0:h], big3[:, :, 0:h], big3[:, :, h:wdt],
                        ALU.add,
                    )
                    wdt = h
                if final_scalar is None:
                    nc.vector.tensor_tensor(
                        out_ap, big3[:, :, 0], big3[:, :, 1], ALU.add
                    )
                else:
                    nc.vector.scalar_tensor_tensor(
                        out_ap, big3[:, :, 0], final_scalar, big3[:, :, 1],
                        ALU.add, ALU.add,
                    )

            def emit_dot_batch(b, h0, h1):
                """dot for rows r in [h0, h1) ∩ [1, n_out]; uses xn block b
                and its S_sb tile; writes sim rows and DMAs out."""
                r0 = max(h0, 1)
                r1 = min(h1, n_out_rows + 1)  # r <= 128
                if r0 >= r1:
                    return
                k0 = r0 - h0
                k1 = r1 - h0
                n0, n1 = k0 * NJ, k1 * NJ
                xbf = xn_tiles[b].rearrange("p r j c -> p (r j) c")
                sbf = sb_tiles.pop(b).rearrange("p r j c -> p (r j) c")
                pd = pdpool.tile([128, RB * NJ, C], BF16, tag="pd", name="pd")
                nc.vector.tensor_tensor(
                    pd[:, n0:n1], xbf[:, n0:n1], sbf[:, n0:n1], ALU.mult
                )
                simo = simpool.tile([128, RB, NJ], F32, tag="simo", name="simo")
                simof = simo.rearrange("p r j -> p (r j)")
                # final level fuses the "- 1" (self-similarity) term
                tree_levels(pd[:, n0:n1], C, simof[:, n0:n1], final_scalar=-1.0)
                nc.sync.dma_start(
                    out_dram[r0 - 1 : r1 - 1].rearrange("r p j -> p r j"),
                    simo[:, k0:k1],
                )

            for b, (h0, h1) in enumerate(blocks):
                R = h1 - h0

                xt = xpool.tile([128, RB, NJ, C], BF16, tag="xt", name="xt")
                nc.sync.dma_start(
                    xt[:, 0:R], x_dram[h0:h1].rearrange("r p f -> p r f")
                )

                # ---- fused square + first tree level -> sq [128, R*NJ, 64]
                n = R * NJ
                xtf = xt.rearrange("p r j c -> p (r j) c")
                sq = sqpool.tile([128, RB * NJ, C // 2], BF16, tag="sq", name="sq")
                nc.vector._custom_dve(
                    SQOP,
                    out=sq[:, 0:n],
                    in0=xtf[:, 0:n, 0 : C // 2],
                    in1=xtf[:, 0:n, C // 2 : C],
                )
                ssr = npool.tile([128, RB, NJ], F32, tag="ssr", name="ssr")
                ssrf = ssr.rearrange("p r j -> p (r j)")
                tree_levels(sq[:, 0:n], C // 2, ssrf[:, 0:n])

                # ---- inv = 1/sqrt(ss + 1e-16), batched
                snorm = npool.tile([128, RB, NJ], F32, tag="snorm", name="snorm")
                nc.scalar.activation(
                    snorm[:, 0:R], ssr[:, 0:R], ACTF.Sqrt, bias=eps_bias[:]
                )
                sinv = npool.tile([128, RB, NJ], F32, tag="sinv", name="sinv")
                nc.vector.reciprocal(sinv[:, 0:R], snorm[:, 0:R])

                # ---- xn chunks
                xnb = xnpool.tile([128, RB, NJ, C], BF16, tag="xn", name="xn")
                xn_tiles[b] = xnb
                sb_tiles[b] = sbpool.tile(
                    [128, RB, NJ, C], BF16, tag="sbt", name="sbt"
                )

                for k in range(R):
                    h = h0 + k
                    for j in range(NJ):
                        if j >= NJ - XN_ACT_CHUNKS:
                            nc.scalar.activation(
                                xnb[:, k, j, :],
                                xt[:, k, j, :],
                                ACTF.Copy,
                                bias=0.0,
                                scale=sinv[:, k, j : j + 1],
                            )
                        else:
                            nc.vector.tensor_scalar(
                                xnb[:, k, j, :],
                                xt[:, k, j, :],
                                sinv[:, k, j : j + 1],
                                None,
                                ALU.mult,
                            )
                    xnt = xnb[:, k]

                    # ---- PE vertical-fold band matmuls (close-early order)
                    r = h - 1
                    if 1 <= r <= n_in - 2:
                        band_matmuls(s_psum[r], xnt, start=False, stop=True)
                        # evac into the owning block's S_sb slice
                        rb = r // RB
                        kk = r - rb * RB
                        nc.scalar.activation(
                            sb_tiles[rb][:, kk], s_psum[r][:], ACTF.Copy
                        )
                        s_psum[r] = None
                        # if this closed the last (valid) row of block rb,
                        # emit its dot batch
                        bh0, bh1 = blocks[rb]
                        if r == min(bh1 - 1, n_out_rows):
                            emit_dot_batch(rb, bh0, bh1)

                    if 1 <= h <= n_in - 2:
                        band_matmuls(s_psum[h], xnt, start=False, stop=False)

                    if 1 <= h + 1 <= n_in - 2:
                        assert s_psum[h + 1] is None
                        s_psum[h + 1] = psumpool.tile(
                            [128, NJ, C], F32, tag="S", name="S"
                        )
                        band_matmuls(s_psum[h + 1], xnt, start=True, stop=False)

                # free xn of block b-2 (consumed by its dot batch by now)
                xn_tiles.pop(b - 2, None)

    nc.compile()
    return nc


def _overlap_gather_full(x):
    xb = x.astype(ml_dtypes.bfloat16)
    WPAD = WSTRIDE * (NJ - 1) + 128 + 8
    xw = np.zeros((H, WPAD, C), ml_dtypes.bfloat16)
    xw[:, 1 : 1 + W] = xb
    p = np.arange(128)[:, None]
    j = np.arange(NJ)[None, :]
    widx = WSTRIDE * j + p
    xov = xw[:, widx, :]  # [H, 128, NJ, C]
    out = np.zeros((H + 2, 128, NJ * C), ml_dtypes.bfloat16)
    out[1 : H + 1] = xov.reshape(H, 128, NJ * C)
    return out


def shard_inputs(input_image):
    x = np.asarray(input_image).reshape(H, W, C).astype(np.float32, copy=False)
    xpad = _overlap_gather_full(x)
    band = build_consts()
    in_maps = []
    for core in range(NCORES):
        lo = core * ROWS_PER_CORE
        shard = np.ascontiguousarray(xpad[lo : lo + N_IN])
        in_maps.append({"x": shard, "band": band})
    return in_maps


def unshard_output(results):
    """results[i]['out'] [128, 128, NJ] (rows, p, j) -> [H, W] f32."""
    w = np.arange(W)
    jw = np.minimum(w // WSTRIDE, NJ - 1)
    pw = w - WSTRIDE * jw + 1
    out = np.empty((H, W), np.float32)
    for core in range(NCORES):
        st = np.asarray(results[core]["out"])  # [rows, 128, NJ]
        h0 = core * ROWS_PER_CORE
        out[h0 : h0 + ROWS_PER_CORE] = st[:, pw, jw]
    return out


_NC_CACHE = {}


def get_nc():
    if "nc" not in _NC_CACHE:
        _NC_CACHE["nc"] = build_bass()
    return _NC_CACHE["nc"]


def kernel(input_image):
    nc = get_nc()
    in_maps = shard_inputs(input_image)
    res = run_bass_kernel_spmd(nc, in_maps, list(range(NCORES)))
    return unshard_output(res.results)


if __name__ == "__main__":
    rng = np.random.default_rng(0)
    x = rng.standard_normal((H, W, 1, C), dtype=np.float32)
    out = kernel(x)
    print(out.shape, out.dtype, out[:2, :4])
